# revision 11
# baseline (speedup 1.0000x reference)
"""Bidirectional Chamfer loss on 8 Trainium2 NeuronCores — banded edition.

Math: for each batch pair (p, q):
    D[i, j] = ||p_i||^2 + ||q_j||^2 - 2 p_i . q_j
    cd = mean_i min_j D[i, j] + mean_j min_i D[i, j]
    loss = 0.7 * mean_b cd_filtered + 0.3 * mean_b cd_nonfiltered

Key idea vs the dense version: both clouds are sorted hierarchically
(x-slabs -> y-runs -> z) so that contiguous index ranges are spatially
local 3D boxes.  For every pred point i (and gt point j) the host computes
a cheap-but-valid upper bound on its NN distance via a 3D grid hash; a
(gt-chunk jt [128 pts] x pred-subrange s [512 cols]) tile is computed ON
DEVICE only if some point's NN-ball can reach the other box.  This is
EXACT (bounds are true upper bounds) and empirically drops ~6x of the
335M-element distance matrix.

Device pipeline per tile (SPMD, same program all 8 cores, per-core data):
    PE   : ldweights (gt chunk [K24,128] split-bf16) + matmul N=512 -> PSUM
    ACT  : scalar.copy PSUM fp32 -> bf16 SBUF group buffer (only ACT drains
           PSUM cheaply; DVE tensor ops from PSUM are 1x)
    DVE  : pacc[slot] = min(cp, pacc[slot])            (2x bf16 TT)
           fold1 over a GROUP of tiles (one strided TT) then one staged
           tensor_reduce -> per-tile gt-chunk mins (m2 cols)
    PE   : per-slot epilogue transpose of pacc + DVE reduce -> pred-side
           mins (m1 cols)
Host: min-combines m2 by chunk id / m1 by subrange (schedule tables),
then means.  Work is bin-packed across cores; slots are padded with
DUPLICATE real chunks (min over duplicates is unchanged) so the program
is identical on every core.
"""

import numpy as np

B = 4
NF = 4096
NN = 8192
NCORES = 8
CH = 128            # gt chunk (PSUM partition dim)
SUB = 512           # pred subrange width (matmul free dim)
K24 = 24            # contraction rows of the split-bf16 matmul
GRP = 8             # tiles per gt-side fold/reduce group

_CACHE = {}


# --------------------------------------------------------------------------
# host-side schedule construction
# --------------------------------------------------------------------------

def _hier_sort(pts, nx, ny):
    """Permutation ordering pts hierarchically x->y->z with equal counts."""
    N = len(pts)
    ix = np.argsort(pts[:, 0], kind="stable")
    out = np.empty(N, dtype=np.int64)
    per_slab = N // nx
    for sx in range(nx):
        slab = ix[sx * per_slab:(sx + 1) * per_slab]
        iy = slab[np.argsort(pts[slab, 1], kind="stable")]
        per_run = per_slab // ny
        for sy in range(ny):
            run = iy[sy * per_run:(sy + 1) * per_run]
            iz = run[np.argsort(pts[run, 2], kind="stable")]
            out[sx * per_slab + sy * per_run:sx * per_slab + (sy + 1) * per_run] = iz
    return out


def _grid_bounds(p, q, h):
    """For each p_i an upper bound on min_j ||p_i - q_j|| via a 3D grid hash
    over q (cell side h).  Searches expanding shells; after the first hit it
    searches one extra shell so the bound is reasonably tight.  Vectorized
    over the common case (hit in the 3x3x3 neighborhood)."""
    from collections import defaultdict

    qc = np.floor(q / h).astype(np.int64)
    cells = defaultdict(list)
    for idx in range(len(q)):
        cells[(qc[idx, 0], qc[idx, 1], qc[idx, 2])].append(idx)
    cells = {k: np.asarray(v) for k, v in cells.items()}
    pc = np.floor(p / h).astype(np.int64)
    out = np.empty(len(p))
    for i in range(len(p)):
        c = (pc[i, 0], pc[i, 1], pc[i, 2])
        best = np.inf
        r = 0
        hit_r = None
        while True:
            # shell at radius r
            cand = []
            if r == 0:
                a = cells.get(c)
                if a is not None:
                    cand.append(a)
            else:
                for dx in range(-r, r + 1):
                    for dy in range(-r, r + 1):
                        if max(abs(dx), abs(dy)) == r:
                            zr = range(-r, r + 1)
                        else:
                            zr = (-r, r)
                        for dz in zr:
                            a = cells.get((c[0] + dx, c[1] + dy, c[2] + dz))
                            if a is not None:
                                cand.append(a)
            if cand:
                idx = np.concatenate(cand)
                d = np.sqrt(((q[idx] - p[i]) ** 2).sum(axis=1)).min()
                best = min(best, d)
                if hit_r is None:
                    hit_r = r
            if hit_r is not None and (r >= hit_r + 1 or best <= (r) * h):
                # all points outside shell r are at distance > r*h... actually
                # points in shells > r are at distance >= (r-?)*h; one extra
                # shell after the first hit is a safe tightening heuristic --
                # the value is an upper bound regardless (distance to a real
                # point); extra shells only tighten it.
                break
            r += 1
        out[i] = best
    return out


def _ball_box_overlap(centers, radii, blo, bhi):
    cl = np.maximum(blo[None, :] - centers, 0.0)
    ch = np.maximum(centers - bhi[None, :], 0.0)
    dist2 = (np.maximum(cl, ch) ** 2).sum(axis=1)
    return dist2 <= radii ** 2


def _need_matrix(p, q, bp, bq, njt, nsub):
    sub_lo = p.reshape(nsub, SUB, 3).min(axis=1)
    sub_hi = p.reshape(nsub, SUB, 3).max(axis=1)
    ch_lo = q.reshape(njt, CH, 3).min(axis=1)
    ch_hi = q.reshape(njt, CH, 3).max(axis=1)
    need = np.zeros((njt, nsub), dtype=bool)
    for s in range(nsub):
        g = _ball_box_overlap(q, bq, sub_lo[s], sub_hi[s]).reshape(njt, CH)
        need[:, s] |= g.any(axis=1)
    for jt in range(njt):
        pr = _ball_box_overlap(p, bp, ch_lo[jt], ch_hi[jt]).reshape(nsub, SUB)
        need[jt, :] |= pr.any(axis=1)
    return need


def build_schedule(inputs):
    """Sort clouds, compute need matrices, bin-pack tiles across cores.

    Returns a dict with, per cfg in ("f", "n"):
      perm_p/perm_q [B, N]      sorting permutations
      nslots [B]                slots per (b) (same for every core)
      caps   [B][nslots]        tile count per slot (same for every core)
      slot_sub [B][core][slot]  subrange id each slot serves (per-core data)
      tiles  [B][core] list of (chunk_id, slot_idx, is_real) in issue order
    """
    cfgs = {}
    for cfg, pk, qk, N, nx, ny in (
        ("f", "pred_filtered", "gt_filtered", NF, 2, 2),
        ("n", "pred_nonfiltered", "gt_nonfiltered", NN, 4, 4),
    ):
        P = np.asarray(inputs[pk], dtype=np.float64)
        Q = np.asarray(inputs[qk], dtype=np.float64)
        njt = N // CH
        nsub = N // SUB
        h = 3.0 * N ** (-1.0 / 3.0)
        perm_p = np.empty((B, N), dtype=np.int64)
        perm_q = np.empty((B, N), dtype=np.int64)
        needs = []
        for b in range(B):
            perm_p[b] = _hier_sort(P[b], nx, ny)
            perm_q[b] = _hier_sort(Q[b], nx, ny)
            p = P[b][perm_p[b]]
            q = Q[b][perm_q[b]]
            bp = _grid_bounds(p, q, h)
            bq = _grid_bounds(q, p, h)
            needs.append(_need_matrix(p, q, bp, bq, njt, nsub))

        # bin-pack: contiguous-run split of (sub, chunklist) across cores
        nslots_b, caps_b, slot_sub_b, tiles_b = [], [], [], []
        for b in range(B):
            need = needs[b]
            lists = [np.nonzero(need[:, s])[0] for s in range(nsub)]
            T = int(sum(len(l) for l in lists))
            # try several slot counts; split lists into pieces <= target,
            # sort desc, slot i serves pieces ranked [8i, 8i+8)
            best = None
            for nsl in range(1, 6):
                tgt = max(1, -(-T // (NCORES * nsl)))
                for C in range(tgt, tgt + 6):
                    pieces_flat = []
                    for s in range(nsub):
                        l = lists[s]
                        for pos in range(0, len(l), C):
                            pieces_flat.append((s, l[pos:pos + C]))
                    if len(pieces_flat) > NCORES * nsl:
                        continue
                    pieces_flat.sort(key=lambda x: -len(x[1]))
                    caps_try = [
                        max((len(pieces_flat[r][1])
                             for r in range(8 * i, min(8 * i + 8, len(pieces_flat)))),
                            default=0)
                        for i in range(nsl)]
                    tot = NCORES * sum(caps_try)
                    if best is None or tot < best[0]:
                        best = (tot, nsl, caps_try, pieces_flat)
                    break
            tot, nslots, caps, pieces_flat = best
            caps = [c for c in caps if c > 0]
            nslots = len(caps)
            pieces = [[] for _ in range(NCORES)]
            for r, pe in enumerate(pieces_flat):
                i, k = r // NCORES, r % NCORES
                while len(pieces[k]) < i:
                    pieces[k].append((pe[0], np.asarray([], dtype=np.int64)))
                pieces[k].append(pe)
            # emit per-core padded tile lists
            slot_sub = np.zeros((NCORES, nslots), dtype=np.int64)
            tiles = [[] for _ in range(NCORES)]
            for k in range(NCORES):
                pc = pieces[k]
                for i in range(nslots):
                    if i < len(pc):
                        s, l = pc[i]
                    elif pc:
                        s, l = pc[0]
                    else:
                        s, l = 0, np.asarray([0])
                    slot_sub[k, i] = s
                    for t in range(caps[i]):
                        if i < len(pc) and t < len(pc[i][1]):
                            tiles[k].append((int(pc[i][1][t]), i, True))
                        else:
                            # dummy: duplicate a real chunk (min unchanged)
                            tiles[k].append((int(l[0]) if len(l) else 0, i, False))
            nslots_b.append(nslots)
            caps_b.append(caps)
            slot_sub_b.append(slot_sub)
            tiles_b.append(tiles)
        cfgs[cfg] = dict(perm_p=perm_p, perm_q=perm_q, nslots=nslots_b,
                         caps=caps_b, slot_sub=slot_sub_b, tiles=tiles_b,
                         njt=njt, nsub=nsub)
    return cfgs


# --------------------------------------------------------------------------
# split-bf16 packing
# --------------------------------------------------------------------------

def _split3(x):
    import ml_dtypes
    bf = ml_dtypes.bfloat16
    b0 = x.astype(bf)
    r1 = (x - b0.astype(np.float32)).astype(np.float32)
    b1 = r1.astype(bf)
    r2 = (r1 - b1.astype(np.float32)).astype(np.float32)
    b2 = r2.astype(bf)
    return b0, b1, b2


_PAIRS = ((0, 0), (0, 1), (1, 0), (0, 2), (1, 1), (2, 0))


def _mk_operands(p, q):
    """p [n,3] pred pts, q [m,3] gt pts -> P [24,n], G [24,m] bf16 such that
    G[:,j] . P[:,i] ~= D[i,j]."""
    import ml_dtypes
    bf = ml_dtypes.bfloat16
    p = p.astype(np.float32)
    q = q.astype(np.float32)
    P = np.zeros((K24, p.shape[0]), bf)
    G = np.zeros((K24, q.shape[0]), bf)
    pp = np.sum(p * p, axis=-1, dtype=np.float32)
    qq = np.sum(q * q, axis=-1, dtype=np.float32)
    for c in range(3):
        ws = _split3(-2.0 * p[:, c])
        gs = _split3(q[:, c])
        for t, (gi, wi) in enumerate(_PAIRS):
            G[6 * c + t, :] = gs[gi]
            P[6 * c + t, :] = ws[wi]
    qqs = _split3(qq)
    pps = _split3(pp)
    for t in range(3):
        G[18 + t, :] = qqs[t]
        P[18 + t, :] = np.ones_like(pp, dtype=bf)
        G[21 + t, :] = np.ones_like(qq, dtype=bf)
        P[21 + t, :] = pps[t]
    return P, G


def pack_inputs(inputs, sched):
    """Build per-core input dicts (gathered operands per schedule)."""
    in_maps = [dict() for _ in range(NCORES)]
    for cfg, pk, qk in (("f", "pred_filtered", "gt_filtered"),
                        ("n", "pred_nonfiltered", "gt_nonfiltered")):
        sc = sched[cfg]
        P = np.asarray(inputs[pk])
        Q = np.asarray(inputs[qk])
        Tb = [len(sc["tiles"][b][0]) for b in range(B)]           # same across cores
        NSb = [sc["nslots"][b] for b in range(B)]
        import ml_dtypes
        bf = ml_dtypes.bfloat16
        gts = [np.zeros((K24, sum(Tb) * CH), dtype=bf) for _ in range(NCORES)]
        prs = [np.zeros((K24, sum(NSb) * SUB), dtype=bf) for _ in range(NCORES)]
        for b in range(B):
            p = P[b][sc["perm_p"][b]]
            q = Q[b][sc["perm_q"][b]]
            Pop, Gop = _mk_operands(p, q)
            t0 = sum(Tb[:b])
            s0 = sum(NSb[:b])
            for k in range(NCORES):
                for i in range(NSb[b]):
                    s = sc["slot_sub"][b][k][i]
                    prs[k][:, (s0 + i) * SUB:(s0 + i + 1) * SUB] = \
                        Pop[:, s * SUB:(s + 1) * SUB]
                for t, (jt, slot, real) in enumerate(sc["tiles"][b][k]):
                    gts[k][:, (t0 + t) * CH:(t0 + t + 1) * CH] = \
                        Gop[:, jt * CH:(jt + 1) * CH]
        for k in range(NCORES):
            in_maps[k]["g" + cfg] = np.ascontiguousarray(gts[k])
            in_maps[k]["p" + cfg] = np.ascontiguousarray(prs[k])
    return in_maps


# --------------------------------------------------------------------------
# device program
# --------------------------------------------------------------------------

def build_nc(sched):
    from contextlib import ExitStack
    import concourse.mybir as mybir
    import concourse.tile as tile
    from concourse import bacc
    from concourse.masks import make_identity

    f32 = mybir.dt.float32
    bf16 = mybir.dt.bfloat16
    Alu = mybir.AluOpType

    Tb = {c: [len(sched[c]["tiles"][b][0]) for b in range(B)] for c in "fn"}
    NSb = {c: [sched[c]["nslots"][b] for b in range(B)] for c in "fn"}
    Ttot = {c: sum(Tb[c]) for c in "fn"}
    NStot = {c: sum(NSb[c]) for c in "fn"}
    n_m2 = Ttot["f"] + Ttot["n"]
    n_m1 = 4 * (NStot["f"] + NStot["n"])

    nc = bacc.Bacc("TRN2", target_bir_lowering=False, debug=False)

    Gf = nc.dram_tensor("gf", [K24, Ttot["f"] * CH], bf16, kind="ExternalInput").ap()
    Pf = nc.dram_tensor("pf", [K24, NStot["f"] * SUB], bf16, kind="ExternalInput").ap()
    Gn = nc.dram_tensor("gn", [K24, Ttot["n"] * CH], bf16, kind="ExternalInput").ap()
    Pn = nc.dram_tensor("pn", [K24, NStot["n"] * SUB], bf16, kind="ExternalInput").ap()
    O1 = nc.dram_tensor("m1", [128, n_m1], f32, kind="ExternalOutput").ap()
    O2 = nc.dram_tensor("m2", [128, n_m2], f32, kind="ExternalOutput").ap()

    with tile.TileContext(nc) as tc, ExitStack() as ctx:
        const_pool = ctx.enter_context(tc.tile_pool(name="const", bufs=1))
        op_pool = ctx.enter_context(tc.tile_pool(name="ops", bufs=1))
        cpg_pool = ctx.enter_context(tc.tile_pool(name="cpg", bufs=4))
        stg_pool = ctx.enter_context(tc.tile_pool(name="stg", bufs=3))
        pacc_pool = ctx.enter_context(tc.tile_pool(name="pacc", bufs=2))
        out_pool = ctx.enter_context(tc.tile_pool(name="outs", bufs=1))
        ps_pool = ctx.enter_context(tc.tile_pool(name="ps", bufs=7, space="PSUM"))
        epi_pool = ctx.enter_context(tc.tile_pool(name="epi", bufs=1, space="PSUM"))

        ident = const_pool.tile([128, 128], bf16)
        make_identity(nc, ident)
        m1t = out_pool.tile([128, n_m1], f32)
        m2t = out_pool.tile([128, n_m2], f32)

        sbo = {}
        for cfg, Gt, Pt in (("f", Gf, Pf), ("n", Gn, Pn)):
            sg = op_pool.tile([K24, Ttot[cfg] * CH], bf16, tag=f"g{cfg}")
            nc.sync.dma_start(sg[:], Gt)
            sp = op_pool.tile([K24, NStot[cfg] * SUB], bf16, tag=f"p{cfg}")
            nc.sync.dma_start(sp[:], Pt)
            sbo[cfg] = (sg, sp)

        m1c = 0
        m2c = 0
        for cfg in ("f", "n"):
            sg, sp = sbo[cfg]
            for b in range(B):
                T = Tb[cfg][b]
                t_base = sum(Tb[cfg][:b])
                s_base = sum(NSb[cfg][:b])
                caps = sched[cfg]["caps"][b]
                nslots = NSb[cfg][b]
                # slot index of each tile position (static across cores)
                slot_of = []
                for i, c in enumerate(caps):
                    slot_of += [i] * c
                assert len(slot_of) == T
                paccs = []
                for i in range(nslots):
                    pacc = pacc_pool.tile([128, SUB], bf16, tag=f"pacc{i}")
                    paccs.append(pacc)
                # group tiles for the gt-side fold/reduce
                t = 0
                groups = []
                while t < T:
                    g = min(GRP, T - t)
                    # shrink to power-of-2-ish tail for clean APs
                    groups.append((t, g))
                    t += g
                for (g0, glen) in groups:
                    cpg = cpg_pool.tile([128, GRP * SUB], bf16, tag="cpg")
                    for u in range(glen):
                        t = g0 + u
                        ps = ps_pool.tile([128, SUB], f32, tag="ps")
                        nc.tensor.matmul(
                            ps[:],
                            lhsT=sg[:, (t_base + t) * CH:(t_base + t + 1) * CH],
                            rhs=sp[:, (s_base + slot_of[t]) * SUB:
                                   (s_base + slot_of[t] + 1) * SUB],
                            start=True, stop=True,
                        )
                        cp = cpg[:, u * SUB:(u + 1) * SUB]
                        nc.scalar.copy(cp, ps[:])
                        # pred-side accumulate
                        sl = slot_of[t]
                        if t == slot_of.index(sl):
                            nc.vector.tensor_copy(paccs[sl][:], cp)
                        else:
                            nc.vector.tensor_tensor(
                                out=paccs[sl][:], in0=cp, in1=paccs[sl][:],
                                op=Alu.min)
                    # gt-side: fold halves of each tile in one strided TT,
                    # then one staged reduce -> glen m2 columns
                    stg = stg_pool.tile([128, GRP * (SUB // 2)], bf16, tag="stg")
                    half = SUB // 2
                    quar = SUB // 4
                    nc.vector.tensor_tensor(
                        out=stg[:, :glen * half].rearrange("p (g k) -> p g k", k=half),
                        in0=cpg[:, :glen * SUB].rearrange(
                            "p (g k) -> p g k", k=SUB)[:, :, :half],
                        in1=cpg[:, :glen * SUB].rearrange(
                            "p (g k) -> p g k", k=SUB)[:, :, half:],
                        op=Alu.min,
                    )
                    stg2 = stg_pool.tile([128, GRP * quar], bf16, tag="stg2")
                    nc.vector.tensor_tensor(
                        out=stg2[:, :glen * quar].rearrange("p (g k) -> p g k", k=quar),
                        in0=stg[:, :glen * half].rearrange(
                            "p (g k) -> p g k", k=half)[:, :, :quar],
                        in1=stg[:, :glen * half].rearrange(
                            "p (g k) -> p g k", k=half)[:, :, quar:],
                        op=Alu.min,
                    )
                    eig = SUB // 8
                    stg3 = stg_pool.tile([128, GRP * eig], bf16, tag="stg3")
                    nc.vector.tensor_tensor(
                        out=stg3[:, :glen * eig].rearrange("p (g k) -> p g k", k=eig),
                        in0=stg2[:, :glen * quar].rearrange(
                            "p (g k) -> p g k", k=quar)[:, :, :eig],
                        in1=stg2[:, :glen * quar].rearrange(
                            "p (g k) -> p g k", k=quar)[:, :, eig:],
                        op=Alu.min,
                    )
                    nc.vector.tensor_reduce(
                        out=m2t[:, m2c + g0:m2c + g0 + glen],
                        in_=stg3[:, :glen * eig].rearrange("p (g k) -> p g k", k=eig),
                        axis=mybir.AxisListType.X,
                        op=Alu.min,
                    )
                m2c += T
                # m1 epilogue per slot: partition-min of pacc via transpose
                for i in range(nslots):
                    ep = epi_pool.tile([128, SUB], bf16, tag="epi")
                    for c in range(SUB // 128):
                        nc.tensor.transpose(
                            ep[:, c * 128:(c + 1) * 128],
                            paccs[i][:, c * 128:(c + 1) * 128],
                            ident,
                        )
                    nc.vector.tensor_reduce(
                        out=m1t[:, m1c:m1c + SUB // 128],
                        in_=ep[:].rearrange("p (c k) -> p c k", k=128),
                        axis=mybir.AxisListType.X,
                        op=Alu.min,
                    )
                    m1c += SUB // 128

        nc.sync.dma_start(O1[:], m1t[:])
        nc.sync.dma_start(O2[:], m2t[:])

    nc.compile()
    return nc


# --------------------------------------------------------------------------
# combine
# --------------------------------------------------------------------------

def combine_outputs(results, sched):
    m1c = 0
    m2c = 0
    cds = {}
    for cfg, N in (("f", NF), ("n", NN)):
        sc = sched[cfg]
        njt = sc["njt"]
        pred_min = {}   # (b, sub) -> [SUB] mins (min-combined over cores)
        gt_min = {}     # (b, jt) -> [CH] mins
        for b in range(B):
            T = len(sc["tiles"][b][0])
            nslots = sc["nslots"][b]
            for k in range(NCORES):
                m2 = results[k]["m2"][:, m2c:m2c + T]        # [128, T]
                for t, (jt, slot, real) in enumerate(sc["tiles"][b][k]):
                    if not real:
                        continue
                    key = (b, jt)
                    v = m2[:, t]
                    if key in gt_min:
                        gt_min[key] = np.minimum(gt_min[key], v)
                    else:
                        gt_min[key] = v.copy()
                m1 = results[k]["m1"][:, m1c:m1c + 4 * nslots]  # [128, 4*nslots]
                # m1[p, i*4+c] = min over partitions of pacc col (c*128+p)
                has_real = [False] * nslots
                for (jt, slot, real) in sc["tiles"][b][k]:
                    if real:
                        has_real[slot] = True
                for i in range(nslots):
                    if not has_real[i]:
                        continue
                    s = sc["slot_sub"][b][k][i]
                    vals = np.empty(SUB, dtype=np.float32)
                    for c in range(4):
                        vals[c * 128:(c + 1) * 128] = m1[:, i * 4 + c]
                    key = (b, s)
                    if key in pred_min:
                        pred_min[key] = np.minimum(pred_min[key], vals)
                    else:
                        pred_min[key] = vals
            m2c += T
            m1c += 4 * nslots
        # means
        cd = np.zeros(B, dtype=np.float64)
        nsub = sc["nsub"]
        for b in range(B):
            pv = np.concatenate([pred_min[(b, s)] for s in range(nsub)])
            gv = np.concatenate([gt_min[(b, jt)] for jt in range(njt)])
            assert pv.shape[0] == N and gv.shape[0] == N
            cd[b] = pv.astype(np.float64).mean() + gv.astype(np.float64).mean()
        cds[cfg] = cd.mean()
    return np.float32(0.7 * cds["f"] + 0.3 * cds["n"])


# --------------------------------------------------------------------------
# entry point
# --------------------------------------------------------------------------

def _sched_sig(sched):
    parts = []
    for cfg in "fn":
        parts.append(tuple(sched[cfg]["nslots"]))
        parts.append(tuple(tuple(c) for c in sched[cfg]["caps"]))
    return tuple(parts)


def kernel(pred_filtered, gt_filtered, pred_nonfiltered, gt_nonfiltered):
    from concourse.bass_utils import run_bass_kernel_spmd

    inputs = dict(pred_filtered=pred_filtered, gt_filtered=gt_filtered,
                  pred_nonfiltered=pred_nonfiltered,
                  gt_nonfiltered=gt_nonfiltered)
    if "sched" not in _CACHE:
        _CACHE["sched"] = build_schedule(inputs)
        _CACHE["sig"] = _sched_sig(_CACHE["sched"])
        _CACHE["nc"] = build_nc(_CACHE["sched"])
    else:
        sched = build_schedule(inputs)
        if _sched_sig(sched) != _CACHE["sig"]:
            _CACHE["sched"] = sched
            _CACHE["sig"] = _sched_sig(sched)
            _CACHE["nc"] = build_nc(sched)
        else:
            _CACHE["sched"] = sched
    sched = _CACHE["sched"]
    in_maps = pack_inputs(inputs, sched)
    res = run_bass_kernel_spmd(_CACHE["nc"], in_maps, core_ids=list(range(NCORES)))
    return combine_outputs(res.results, sched)
